# revision 7
# baseline (speedup 1.0000x reference)
"""AAFM sparse-attention kernel for 8 TRN2 NeuronCores.

Math (per batch b):
    qp = q @ Wq.T + bq ; kp = k @ Wk.T (+bk) ; vp = v @ Wv.T (+bv)
    q_sig = sigmoid(qp)
    exp_a = exp(-alpha * log2(Sk) * distances)        # [Sq, Sk]
    exp_k = exp(kp)                                   # [Sk, D]
    out   = q_sig * (exp_a @ (exp_k * vp)) / (exp_a @ exp_k)

Algebraic simplifications (exact in real arithmetic):
  - bk cancels: exp(kp+bk) = exp(kp)*exp(bk) factors out of num and den.
  - bv pulls out: att = num'/den + bv, applied as a cheap epilogue add.

Precision split (validated numerically, gate rel<2e-2):
  - numerator  A@(ek*vp): bf16 (errors are random-sign sums -> full strength)
  - denominator A@ek: fp8e4 DoubleRow at 2x PE throughput. den is an
    all-positive weighted sum, so elementwise fp8 noise averages down by
    ~1/sqrt(n_eff); simulated end-to-end rel err 3.8e-3.
    fp8e4 (TRN: max 240): ek in [~0.05, 13] and ea in (0,1] fit raw.

Sharding: data-parallel over batch B=8, one batch per core; no collectives.

Per-core structure:
  Phase A (16 s-tiles, k/v only): 8 projection MMs (K=128,N=512) per tile
    via bf16 casts; ScalarE exp(kp) from PSUM -> ek scratch; DVE builds
    resident Bm = 0.5*ek*vp bf16 [128,16,512] and EK8 = ek fp8 [128,16,512].
    q projections are DEFERRED to phase B (cuts phase-A DMA need by 1/3 so
    the HBM window matches the PE window).
  Phase B (16 q-tiles): dT DMA -> ScalarE exp -> ea bf16; DVE cast -> ea8;
    per tile: 4 q-proj MMs (qp -> +bq -> tanh 0.5x = sigmoid), 8 fp8
    DoubleRow den MMs (K=256 each), 16 bf16 num MMs; DVE epilogue
    (tanh+1) * (num*0.5*recip(den) + bv/2), batched out DMA.
DMA: Sync HWDGE ring carries k/v (512KB group loads), then dT (1MB) and
qT slices (256KB); Scalar HWDGE ring carries weights + biases + outputs.
A dummy MM chain on memset tiles warms the PE clock during startup.
"""

import math
import sys

import numpy as np

sys.path.insert(0, "/opt/trn_rl_repo")

import concourse.bass as bass  # noqa: E402
import concourse.tile as tile  # noqa: E402
from concourse import bacc, mybir  # noqa: E402
from concourse.bass_utils import run_bass_kernel_spmd  # noqa: E402

P = 128
D = 512
S = 2048
B = 8
N_CORES = 8
DC = D // P  # 4 contraction chunks for projections

F32 = mybir.dt.float32
BF16 = mybir.dt.bfloat16
F8 = mybir.dt.float8e4
DR = mybir.MatmulPerfMode.DoubleRow
AF = mybir.ActivationFunctionType
ALU = mybir.AluOpType


def build_graph(exp_scale: float, s: int = S):
    """Build the single-core Bass/Tile graph. Same graph runs SPMD on 8 cores."""
    nt = s // P  # s-tiles == k-chunks == q-tiles
    ga = 2  # s-tiles per k/v group DMA (512KB per tensor)
    nc = bacc.Bacc(
        "TRN2",
        target_bir_lowering=False,
        debug=False,
        enable_asserts=True,
        num_devices=N_CORES,
    )

    qT = nc.dram_tensor("qT", [D, s], F32, kind="ExternalInput").ap()
    kT = nc.dram_tensor("kT", [D, s], F32, kind="ExternalInput").ap()
    vT = nc.dram_tensor("vT", [D, s], F32, kind="ExternalInput").ap()
    dT = nc.dram_tensor("dT", [s, s], F32, kind="ExternalInput").ap()
    wq = nc.dram_tensor("wq", [D, D], F32, kind="ExternalInput").ap()
    wk = nc.dram_tensor("wk", [D, D], F32, kind="ExternalInput").ap()
    wv = nc.dram_tensor("wv", [D, D], F32, kind="ExternalInput").ap()
    bq = nc.dram_tensor("bq", [P, D], F32, kind="ExternalInput").ap()
    bv = nc.dram_tensor("bv", [P, D], F32, kind="ExternalInput").ap()
    out = nc.dram_tensor("out", [s, D], F32, kind="ExternalOutput").ap()

    qT_r = qT.rearrange("(c p) s -> p c s", p=P)
    kT_r = kT.rearrange("(c p) s -> p c s", p=P)
    vT_r = vT.rearrange("(c p) s -> p c s", p=P)
    dT_r = dT.rearrange("(c p) q -> p c q", p=P)
    out_r = out.rearrange("(t p) e -> p t e", p=P)

    def mm(ps_ap, lhsT, rhs, start, stop, **kw):
        nc.tensor.matmul(ps_ap, lhsT, rhs, start=start, stop=stop, **kw)

    with tile.TileContext(nc) as tc:
        with (
            tc.tile_pool(name="consts", bufs=1) as consts,
            tc.tile_pool(name="wstage", bufs=3) as wstage,
            tc.tile_pool(name="resident", bufs=1) as resident,
            tc.tile_pool(name="stageA", bufs=3) as stageA,
            tc.tile_pool(name="stageB", bufs=3) as stageB,
            tc.tile_pool(name="stageQ", bufs=2) as stageQ,
            tc.tile_pool(name="tmpA", bufs=2) as tmpA,
            tc.tile_pool(name="tmpB", bufs=2) as tmpB,
            tc.tile_pool(name="outp", bufs=2) as outp,
            tc.tile_pool(name="psA", bufs=2, space="PSUM") as psA,
            tc.tile_pool(name="psB", bufs=2, space="PSUM") as psB,
            tc.tile_pool(name="psQ", bufs=2, space="PSUM") as psQ,
        ):
            # Warm the ACT exp table set + PE clock during startup DMA wait.
            warm = consts.tile([P, D], BF16, tag="warm")
            nc.vector.memset(warm[:], 0.001)
            wexp = consts.tile([P, 1], F32, tag="wexp")
            nc.vector.memset(wexp[:], 0.0)
            nc.scalar.activation(wexp[:], wexp[:], AF.Exp)
            # Bridge the phase-A HBM fill deficit so real MMs run at 2.4 GHz.
            NDUMMY = 16
            wps = psA.tile([P, D], F32, tag="ps")
            for w in range(NDUMMY):
                mm(wps[:], warm[:, 0:P], warm[:], w == 0, w == NDUMMY - 1)

            # Weights: 256KB f32 chunks on Scalar ring, DVE cast to bf16.
            w_sb = {}
            for name, drm in (("wk", wk), ("wv", wv), ("wq", wq)):
                t = consts.tile([P, DC, D], BF16, tag=f"w_{name}")
                drm_r = drm.rearrange("(c p) e -> p c e", p=P)
                for c in range(DC):
                    st = wstage.tile([P, D], F32, tag="wstage")
                    nc.scalar.dma_start(st[:], drm_r[:, c, :])
                    nc.vector.tensor_copy(t[:, c, :], st[:])
                w_sb[name] = t
            bq_sb = consts.tile([P, D], F32, tag="bq")
            nc.scalar.dma_start(bq_sb[:], bq[:])
            bvh = consts.tile([P, D], F32, tag="bvh")
            nc.scalar.dma_start(bvh[:], bv[:])
            nc.vector.tensor_scalar_mul(bvh[:], bvh[:], 0.5)

            # Residents: Bm = 0.5*ek*vp (bf16, num moving operand),
            # EK8 = ek (fp8, den moving operand). k on partitions; chunk i
            # holds rows k = i*128+p.
            Bm = resident.tile([P, nt, D], BF16)
            EK8 = resident.tile([P, nt, D], F8)

            # ---- Phase A: k/v projections, exp_k, Bm/EK8 build ----
            for g in range(nt // ga):
                gsl = bass.ts(g, ga * P)
                kv_f32 = {}
                for nm, src in (("k", kT_r), ("v", vT_r)):
                    t = stageA.tile([P, DC, ga * P], F32, tag=f"{nm}f32")
                    nc.sync.dma_start(t[:], src[:, :, gsl])
                    kv_f32[nm] = t

                for nm, wname in (("k", "wk"), ("v", "wv")):
                    pss = []
                    for ii in range(ga):
                        a = stageA.tile([P, DC, P], BF16, tag=f"{nm}a")
                        nc.vector.tensor_copy(
                            a[:], kv_f32[nm][:, :, bass.ts(ii, P)]
                        )
                        p = psA.tile([P, D], F32, tag="ps")
                        pss.append(p)
                        for c in range(DC):
                            mm(p[:], a[:, c, :], w_sb[wname][:, c, :], c == 0, c == DC - 1)
                    for ii in range(ga):
                        i = g * ga + ii
                        if nm == "k":
                            # ek = exp(kp) from PSUM -> scratch bf16 + fp8 copy
                            eks = tmpA.tile([P, D], BF16, tag="eks")
                            nc.scalar.activation(eks[:], pss[ii][:], AF.Exp)
                            nc.vector.tensor_copy(EK8[:, i, :], eks[:])
                            kv_f32[f"ek{ii}"] = eks
                        else:
                            # Bm = 0.5*ek*vp (0.5 pre-folds the sigmoid half)
                            nc.vector.scalar_tensor_tensor(
                                Bm[:, i, 0:D],
                                kv_f32[f"ek{ii}"][:],
                                0.5,
                                pss[ii][:],
                                op0=ALU.mult,
                                op1=ALU.mult,
                            )

            # Prefetch first phase-B inputs on the sync ring (they queue
            # behind the k/v groups and fill the ring's tail window).
            da_t, ea_t, ea8_t, qf_t = [], [], [], []
            PF = 2

            def issue_da(j):
                da = stageB.tile([P, nt, P], F32, tag="da")
                nc.sync.dma_start(da[:], dT_r[:, :, bass.ts(j, P)])
                da_t.append(da)

            def issue_qf(j):
                qf = stageQ.tile([P, DC, P], F32, tag="qf")
                nc.sync.dma_start(qf[:], qT_r[:, :, bass.ts(j, P)])
                qf_t.append(qf)

            def issue_ea(j):
                da = da_t[j]
                ea = stageB.tile([P, nt, P], BF16, tag="ea")
                nc.scalar.activation(ea[:], da[:], AF.Exp, scale=exp_scale)
                ea8 = stageB.tile([P, nt, P], F8, tag="ea8")
                nc.vector.tensor_copy(ea8[:], ea[:])
                ea_t.append(ea)
                ea8_t.append(ea8)

            with tc.tile_wait_until(0.020):
                for j in range(PF):
                    issue_da(j)
                    issue_qf(j)
            issue_ea(0)

            # ---- Phase B: q proj, exp_a, attention matmuls, epilogue ----
            for j in range(nt):
                if j + PF < nt:
                    issue_da(j + PF)
                    issue_qf(j + PF)
                if j + 1 < nt:
                    issue_ea(j + 1)
                ea, ea8, qf = ea_t[j], ea8_t[j], qf_t[j]

                # q projection for this tile: qp -> +bq -> tanh(x/2)
                qa = stageQ.tile([P, DC, P], BF16, tag="qa")
                nc.vector.tensor_copy(qa[:], qf[:])
                qp = psQ.tile([P, D], F32, tag="qp")
                for c in range(DC):
                    mm(qp[:], qa[:, c, :], w_sb["wq"][:, c, :], c == 0, c == DC - 1)
                qpb = tmpB.tile([P, D], F32, tag="qpb")
                nc.vector.tensor_add(qpb[:], qp[:], bq_sb[:])
                tq = tmpB.tile([P, D], BF16, tag="tq")
                nc.scalar.activation(tq[:], qpb[:], AF.Tanh, scale=0.5)

                ps = psB.tile([P, 2, D], F32, tag="att")
                r = tmpB.tile([P, D], F32, tag="recip")
                rq = tmpB.tile([P, D], F32, tag="rq")
                tqb = tmpB.tile([P, D], F32, tag="tqb")
                # den first (fp8 DoubleRow, K=256 per MM): recip + epilogue
                # prep overlap the num MMs.
                for c in range(nt // 2):
                    mm(
                        ps[:, 1, :],
                        ea8[:, 2 * c : 2 * c + 2, :],
                        EK8[:, 2 * c : 2 * c + 2, :],
                        c == 0,
                        c == nt // 2 - 1,
                        perf_mode=DR,
                    )
                nc.vector.reciprocal_approx_fast(r[:], ps[:, 1, :])
                # rq = (tanh+1)/den ; tqb = (tanh+1) * bv/2
                nc.vector.scalar_tensor_tensor(
                    rq[:], tq[:], 1.0, r[:], op0=ALU.add, op1=ALU.mult
                )
                nc.vector.scalar_tensor_tensor(
                    tqb[:], tq[:], 1.0, bvh[:], op0=ALU.add, op1=ALU.mult
                )
                # num (bf16): PSUM carries the 0.5 fold from phase A
                for c in range(nt):
                    mm(
                        ps[:, 0, :],
                        ea[:, c, :],
                        Bm[:, c, :],
                        c == 0,
                        c == nt - 1,
                    )
                # out = num*rq + tqb  ==  sigmoid(qp) * (num/den + bv)
                na = tmpB.tile([P, D], F32, tag="na")
                nc.vector.tensor_mul(na[:], ps[:, 0, :], rq[:])
                ot = outp.tile([P, D], F32, tag="ot")
                nc.gpsimd.tensor_add(ot[:], na[:], tqb[:])
                nc.scalar.dma_start(out_r[:, j, :], ot[:])

    nc.compile()
    return nc


def make_in_maps(q, k, v, distances, Wq, bq, Wk, bk, Wv, bv):
    """Per-core input maps: layout-only host work (transposes, bias tiling)."""
    wq_t = np.ascontiguousarray(Wq.T)  # [d, e]
    wk_t = np.ascontiguousarray(Wk.T)
    wv_t = np.ascontiguousarray(Wv.T)
    bq_t = np.ascontiguousarray(np.broadcast_to(bq[None, :], (P, D)))
    bv_t = np.ascontiguousarray(np.broadcast_to(bv[None, :], (P, D)))
    in_maps = []
    for b in range(B):
        in_maps.append(
            {
                "qT": np.ascontiguousarray(q[b].T),
                "kT": np.ascontiguousarray(k[b].T),
                "vT": np.ascontiguousarray(v[b].T),
                "dT": np.ascontiguousarray(distances[b].T),
                "wq": wq_t,
                "wk": wk_t,
                "wv": wv_t,
                "bq": bq_t,
                "bv": bv_t,
            }
        )
    return in_maps


def _exp_scale(alpha, n):
    # mirror reference: log2_n = log(n)/log(2) in fp32, bias = -alpha*log2_n*d
    log2_n = np.float32(np.log(np.float32(n))) / np.float32(np.log(np.float32(2.0)))
    return float(np.float32(-np.float32(alpha) * log2_n))


_GRAPH_CACHE = {}


def run(q, k, v, distances, Wq, bq, Wk, bk, Wv, bv, alpha, trace=False, tmpdir=None):
    scale = _exp_scale(alpha[0], k.shape[1])
    key = scale
    if key not in _GRAPH_CACHE:
        _GRAPH_CACHE[key] = build_graph(scale)
    nc = _GRAPH_CACHE[key]
    in_maps = make_in_maps(q, k, v, distances, Wq, bq, Wk, bk, Wv, bv)
    res = run_bass_kernel_spmd(
        nc, in_maps, core_ids=list(range(N_CORES)), trace=trace, tmpdir=tmpdir
    )
    outs = np.stack([res.results[b]["out"] for b in range(B)], axis=0)
    return outs.astype(np.float32), res


def kernel(q, k, v, distances, Wq, bq, Wk, bk, Wv, bv, alpha):
    out, _ = run(q, k, v, distances, Wq, bq, Wk, bk, Wv, bv, alpha, trace=False)
    return out


# revision 8
# speedup vs baseline: 1.0530x; 1.0530x over previous
"""AAFM sparse-attention kernel for 8 TRN2 NeuronCores.

Math (per batch b):
    qp = q @ Wq.T + bq ; kp = k @ Wk.T (+bk) ; vp = v @ Wv.T (+bv)
    q_sig = sigmoid(qp)
    exp_a = exp(-alpha * log2(Sk) * distances)        # [Sq, Sk]
    exp_k = exp(kp)                                   # [Sk, D]
    out   = q_sig * (exp_a @ (exp_k * vp)) / (exp_a @ exp_k)

Algebraic simplifications (exact in real arithmetic):
  - bk cancels: exp(kp+bk) = exp(kp)*exp(bk) factors out of num and den.
  - bv pulls out: att = num'/den + bv, applied as a cheap epilogue add.

Precision split (validated numerically, gate rel<2e-2):
  - numerator  A@(ek*vp): bf16 (errors are random-sign sums -> full strength)
  - denominator A@ek: fp8e4 DoubleRow at 2x PE throughput. den is an
    all-positive weighted sum, so elementwise fp8 noise averages down by
    ~1/sqrt(n_eff); simulated end-to-end rel err 3.8e-3.
    fp8e4 (TRN: max 240): ek in [~0.05, 13] and ea in (0,1] fit raw.

Sharding: data-parallel over batch B=8, one batch per core; no collectives.

Per-core structure:
  Phase A (16 s-tiles, k/v only): 8 projection MMs (K=128,N=512) per tile
    via bf16 casts; ScalarE exp(kp) from PSUM -> ek scratch; DVE builds
    resident Bm = 0.5*ek*vp bf16 [128,16,512] and EK8 = ek fp8 [128,16,512].
    q projections are DEFERRED to phase B (cuts phase-A DMA need by 1/3 so
    the HBM window matches the PE window).
  Phase B (16 q-tiles): dT DMA -> ScalarE exp -> ea bf16; DVE cast -> ea8;
    per tile: 4 q-proj MMs (qp -> +bq -> tanh 0.5x = sigmoid), 8 fp8
    DoubleRow den MMs (K=256 each), 16 bf16 num MMs; DVE epilogue
    (tanh+1) * (num*0.5*recip(den) + bv/2), batched out DMA.
DMA: Sync HWDGE ring carries k/v (512KB group loads), then dT (1MB) and
qT slices (256KB); Scalar HWDGE ring carries weights + biases + outputs.
A dummy MM chain on memset tiles warms the PE clock during startup.
"""

import math
import sys

import numpy as np

sys.path.insert(0, "/opt/trn_rl_repo")

import concourse.bass as bass  # noqa: E402
import concourse.tile as tile  # noqa: E402
from concourse import bacc, mybir  # noqa: E402
from concourse.bass_utils import run_bass_kernel_spmd  # noqa: E402

P = 128
D = 512
S = 2048
B = 8
N_CORES = 8
DC = D // P  # 4 contraction chunks for projections

F32 = mybir.dt.float32
BF16 = mybir.dt.bfloat16
F8 = mybir.dt.float8e4
DR = mybir.MatmulPerfMode.DoubleRow
AF = mybir.ActivationFunctionType
ALU = mybir.AluOpType


def build_graph(exp_scale: float, s: int = S):
    """Build the single-core Bass/Tile graph. Same graph runs SPMD on 8 cores."""
    nt = s // P  # s-tiles == k-chunks == q-tiles
    ga = 2  # s-tiles per k/v group DMA (512KB per tensor)
    nc = bacc.Bacc(
        "TRN2",
        target_bir_lowering=False,
        debug=False,
        enable_asserts=True,
        num_devices=N_CORES,
    )

    qT = nc.dram_tensor("qT", [D, s], F32, kind="ExternalInput").ap()
    kT = nc.dram_tensor("kT", [D, s], F32, kind="ExternalInput").ap()
    vT = nc.dram_tensor("vT", [D, s], F32, kind="ExternalInput").ap()
    dT = nc.dram_tensor("dT", [s, s], F32, kind="ExternalInput").ap()
    wq = nc.dram_tensor("wq", [D, D], F32, kind="ExternalInput").ap()
    wk = nc.dram_tensor("wk", [D, D], F32, kind="ExternalInput").ap()
    wv = nc.dram_tensor("wv", [D, D], F32, kind="ExternalInput").ap()
    bq = nc.dram_tensor("bq", [P, D], F32, kind="ExternalInput").ap()
    bv = nc.dram_tensor("bv", [P, D], F32, kind="ExternalInput").ap()
    out = nc.dram_tensor("out", [s, D], F32, kind="ExternalOutput").ap()

    qT_r = qT.rearrange("(c p) s -> p c s", p=P)
    kT_r = kT.rearrange("(c p) s -> p c s", p=P)
    vT_r = vT.rearrange("(c p) s -> p c s", p=P)
    dT_r = dT.rearrange("(c p) q -> p c q", p=P)
    out_r = out.rearrange("(t p) e -> p t e", p=P)

    def mm(ps_ap, lhsT, rhs, start, stop, **kw):
        nc.tensor.matmul(ps_ap, lhsT, rhs, start=start, stop=stop, **kw)

    with tile.TileContext(nc) as tc:
        with (
            tc.tile_pool(name="consts", bufs=1) as consts,
            tc.tile_pool(name="wstage", bufs=3) as wstage,
            tc.tile_pool(name="resident", bufs=1) as resident,
            tc.tile_pool(name="stageA", bufs=4) as stageA,
            tc.tile_pool(name="stageB", bufs=3) as stageB,
            tc.tile_pool(name="stageQ", bufs=2) as stageQ,
            tc.tile_pool(name="tmpA", bufs=2) as tmpA,
            tc.tile_pool(name="tmpB", bufs=2) as tmpB,
            tc.tile_pool(name="outp", bufs=2) as outp,
            tc.tile_pool(name="psA", bufs=2, space="PSUM") as psA,
            tc.tile_pool(name="psB", bufs=2, space="PSUM") as psB,
            tc.tile_pool(name="psQ", bufs=2, space="PSUM") as psQ,
        ):
            # Warm the ACT exp table set + PE clock during startup DMA wait.
            warm = consts.tile([P, D], BF16, tag="warm")
            nc.vector.memset(warm[:], 0.001)
            wexp = consts.tile([P, 1], F32, tag="wexp")
            nc.vector.memset(wexp[:], 0.0)
            nc.scalar.activation(wexp[:], wexp[:], AF.Exp)
            # Bridge the phase-A HBM fill deficit so real MMs run at 2.4 GHz.
            NDUMMY = 20
            wps = psA.tile([P, D], F32, tag="ps")
            for w in range(NDUMMY):
                mm(wps[:], warm[:, 0:P], warm[:], w == 0, w == NDUMMY - 1)

            # Weights: 256KB f32 chunks on Scalar ring, DVE cast to bf16.
            w_sb = {}
            for name, drm, eng in (
                ("wk", wk, nc.sync),
                ("wv", wv, nc.sync),
                ("wq", wq, nc.scalar),
            ):
                t = consts.tile([P, DC, D], BF16, tag=f"w_{name}")
                drm_r = drm.rearrange("(c p) e -> p c e", p=P)
                for c in range(DC):
                    st = wstage.tile([P, D], F32, tag="wstage")
                    eng.dma_start(st[:], drm_r[:, c, :])
                    nc.vector.tensor_copy(t[:, c, :], st[:])
                w_sb[name] = t
            bq_sb = consts.tile([P, D], F32, tag="bq")
            nc.scalar.dma_start(bq_sb[:], bq[:])
            bvh = consts.tile([P, D], F32, tag="bvh")
            nc.scalar.dma_start(bvh[:], bv[:])
            nc.vector.tensor_scalar_mul(bvh[:], bvh[:], 0.5)

            # Residents: Bm = 0.5*ek*vp (bf16, num moving operand),
            # EK8 = ek (fp8, den moving operand). k on partitions; chunk i
            # holds rows k = i*128+p.
            Bm = resident.tile([P, nt, D], BF16)
            EK8 = resident.tile([P, nt, D], F8)

            # ---- Phase A: k/v projections, exp_k, Bm/EK8 build ----
            for g in range(nt // ga):
                gsl = bass.ts(g, ga * P)
                kv_f32 = {}
                for nm, src in (("k", kT_r), ("v", vT_r)):
                    t = stageA.tile([P, DC, ga * P], F32, tag=f"{nm}f32")
                    nc.sync.dma_start(t[:], src[:, :, gsl])
                    kv_f32[nm] = t

                for nm, wname in (("k", "wk"), ("v", "wv")):
                    pss = []
                    for ii in range(ga):
                        a = stageA.tile([P, DC, P], BF16, tag=f"{nm}a")
                        nc.vector.tensor_copy(
                            a[:], kv_f32[nm][:, :, bass.ts(ii, P)]
                        )
                        p = psA.tile([P, D], F32, tag="ps")
                        pss.append(p)
                        for c in range(DC):
                            mm(p[:], a[:, c, :], w_sb[wname][:, c, :], c == 0, c == DC - 1)
                    for ii in range(ga):
                        i = g * ga + ii
                        if nm == "k":
                            # ek = exp(kp) from PSUM -> scratch bf16 + fp8 copy
                            eks = tmpA.tile([P, D], BF16, tag="eks")
                            nc.scalar.activation(eks[:], pss[ii][:], AF.Exp)
                            nc.vector.tensor_copy(EK8[:, i, :], eks[:])
                            kv_f32[f"ek{ii}"] = eks
                        else:
                            # Bm = 0.5*ek*vp (0.5 pre-folds the sigmoid half)
                            nc.vector.scalar_tensor_tensor(
                                Bm[:, i, 0:D],
                                kv_f32[f"ek{ii}"][:],
                                0.5,
                                pss[ii][:],
                                op0=ALU.mult,
                                op1=ALU.mult,
                            )

            # Prefetch first phase-B inputs on the sync ring (they queue
            # behind the k/v groups and fill the ring's tail window).
            da_t, ea_t, ea8_t, qf_t = [], [], [], []
            PF = 2

            def issue_da(j):
                da = stageB.tile([P, nt, P], F32, tag="da")
                nc.sync.dma_start(da[:], dT_r[:, :, bass.ts(j, P)])
                da_t.append(da)

            def issue_qf(j):
                qf = stageQ.tile([P, DC, P], F32, tag="qf")
                nc.sync.dma_start(qf[:], qT_r[:, :, bass.ts(j, P)])
                qf_t.append(qf)

            def issue_ea(j):
                da = da_t[j]
                ea = stageB.tile([P, nt, P], BF16, tag="ea")
                nc.scalar.activation(ea[:], da[:], AF.Exp, scale=exp_scale)
                ea8 = stageB.tile([P, nt, P], F8, tag="ea8")
                nc.vector.tensor_copy(ea8[:], ea[:])
                ea_t.append(ea)
                ea8_t.append(ea8)

            with tc.tile_wait_until(0.020):
                for j in range(PF):
                    issue_da(j)
                    issue_qf(j)
            issue_ea(0)

            # ---- Phase B: q proj, exp_a, attention matmuls, epilogue ----
            for j in range(nt):
                if j + PF < nt:
                    issue_da(j + PF)
                    issue_qf(j + PF)
                if j + 1 < nt:
                    issue_ea(j + 1)
                ea, ea8, qf = ea_t[j], ea8_t[j], qf_t[j]

                # q projection for this tile: qp -> +bq -> tanh(x/2)
                qa = stageQ.tile([P, DC, P], BF16, tag="qa")
                nc.vector.tensor_copy(qa[:], qf[:])
                qp = psQ.tile([P, D], F32, tag="qp")
                for c in range(DC):
                    mm(qp[:], qa[:, c, :], w_sb["wq"][:, c, :], c == 0, c == DC - 1)
                qpb = tmpB.tile([P, D], F32, tag="qpb")
                nc.vector.tensor_add(qpb[:], qp[:], bq_sb[:])
                tq = tmpB.tile([P, D], BF16, tag="tq")
                nc.scalar.activation(tq[:], qpb[:], AF.Tanh, scale=0.5)

                ps = psB.tile([P, 2, D], F32, tag="att")
                r = tmpB.tile([P, D], F32, tag="recip")
                rq = tmpB.tile([P, D], F32, tag="rq")
                tqb = tmpB.tile([P, D], F32, tag="tqb")
                # den first (fp8 DoubleRow, K=256 per MM): recip + epilogue
                # prep overlap the num MMs.
                for c in range(nt // 2):
                    mm(
                        ps[:, 1, :],
                        ea8[:, 2 * c : 2 * c + 2, :],
                        EK8[:, 2 * c : 2 * c + 2, :],
                        c == 0,
                        c == nt // 2 - 1,
                        perf_mode=DR,
                    )
                nc.vector.reciprocal_approx_fast(r[:], ps[:, 1, :])
                # rq = (tanh+1)/den ; tqb = (tanh+1) * bv/2
                nc.vector.scalar_tensor_tensor(
                    rq[:], tq[:], 1.0, r[:], op0=ALU.add, op1=ALU.mult
                )
                nc.vector.scalar_tensor_tensor(
                    tqb[:], tq[:], 1.0, bvh[:], op0=ALU.add, op1=ALU.mult
                )
                # num (bf16): PSUM carries the 0.5 fold from phase A
                for c in range(nt):
                    mm(
                        ps[:, 0, :],
                        ea[:, c, :],
                        Bm[:, c, :],
                        c == 0,
                        c == nt - 1,
                    )
                # out = num*rq + tqb  ==  sigmoid(qp) * (num/den + bv)
                na = tmpB.tile([P, D], F32, tag="na")
                nc.vector.tensor_mul(na[:], ps[:, 0, :], rq[:])
                ot = outp.tile([P, D], F32, tag="ot")
                nc.vector.tensor_add(ot[:], na[:], tqb[:])
                nc.scalar.dma_start(out_r[:, j, :], ot[:])

    nc.compile()
    return nc


def make_in_maps(q, k, v, distances, Wq, bq, Wk, bk, Wv, bv):
    """Per-core input maps: layout-only host work (transposes, bias tiling)."""
    wq_t = np.ascontiguousarray(Wq.T)  # [d, e]
    wk_t = np.ascontiguousarray(Wk.T)
    wv_t = np.ascontiguousarray(Wv.T)
    bq_t = np.ascontiguousarray(np.broadcast_to(bq[None, :], (P, D)))
    bv_t = np.ascontiguousarray(np.broadcast_to(bv[None, :], (P, D)))
    in_maps = []
    for b in range(B):
        in_maps.append(
            {
                "qT": np.ascontiguousarray(q[b].T),
                "kT": np.ascontiguousarray(k[b].T),
                "vT": np.ascontiguousarray(v[b].T),
                "dT": np.ascontiguousarray(distances[b].T),
                "wq": wq_t,
                "wk": wk_t,
                "wv": wv_t,
                "bq": bq_t,
                "bv": bv_t,
            }
        )
    return in_maps


def _exp_scale(alpha, n):
    # mirror reference: log2_n = log(n)/log(2) in fp32, bias = -alpha*log2_n*d
    log2_n = np.float32(np.log(np.float32(n))) / np.float32(np.log(np.float32(2.0)))
    return float(np.float32(-np.float32(alpha) * log2_n))


_GRAPH_CACHE = {}


def run(q, k, v, distances, Wq, bq, Wk, bk, Wv, bv, alpha, trace=False, tmpdir=None):
    scale = _exp_scale(alpha[0], k.shape[1])
    key = scale
    if key not in _GRAPH_CACHE:
        _GRAPH_CACHE[key] = build_graph(scale)
    nc = _GRAPH_CACHE[key]
    in_maps = make_in_maps(q, k, v, distances, Wq, bq, Wk, bk, Wv, bv)
    res = run_bass_kernel_spmd(
        nc, in_maps, core_ids=list(range(N_CORES)), trace=trace, tmpdir=tmpdir
    )
    outs = np.stack([res.results[b]["out"] for b in range(B)], axis=0)
    return outs.astype(np.float32), res


def kernel(q, k, v, distances, Wq, bq, Wk, bk, Wv, bv, alpha):
    out, _ = run(q, k, v, distances, Wq, bq, Wk, bk, Wv, bv, alpha, trace=False)
    return out
